# revision 2
# baseline (speedup 1.0000x reference)
"""Trainium2 Bass kernel for MultiHeadSelfAttention (K-only variant).

Math (per batch b):
    K  = x @ Wk.T;  Kh = heads(K)
    S_h = Kh @ Kh.T / sqrt(D);  P_h = softmax(S_h)
    wV_h = P_h @ Kh  (V == K);  out = concat_h(wV) @ Wo.T

Sharding (8 cores): core c handles batch c//2 and query-half c%2 with all
heads.  The query half is selected by rolling x on the host so each core
always computes queries 0:S//2 of its (rolled) sequence; softmax over keys
is order-invariant so rolling the key axis is harmless.

Per-core pipeline (one SPMD NEFF):
    xT_bf  = XBAR-DMA-transpose(bf16(x))     [d, s]
    K      = xT_bf.T @ WkT_bf  (bf16 matmuls, fp32 psum) -> k_bf, kones
    khT    = XBAR-DMA-transpose(k_bf)        [e, s] bf16
    per (qb, head):
      scores strip S_T[k, q] = khT_h.T @ khT_h[:, qb]    (bf16)
      E_T = exp(S_T / sqrt(D))       (ScalarE, psum -> sbuf bf16)
      PV:  [wVT_h ; rowsum_h] = [Kh_h | 1].T @ E_T       (bf16, psum accum)
      recip = 1/rowsum; partition-broadcast via K=1 matmul; normalize wVT
    out = wVTn.T @ WoT  (fp32r), first half overlapped with second qb
"""

import sys

if "/opt/trn_rl_repo" not in sys.path:
    sys.path.insert(0, "/opt/trn_rl_repo")

import numpy as np

B, S, D = 4, 2048, 512
H = 8
HD = D // H            # 64
P = 128
SH = S // 2            # query half per core = 1024
NCORES = 8
SCALE = 1.0 / np.sqrt(D)

_CACHE = {}


def _build_nc(repeat: int = 1, mode: str = "full"):
    import concourse.bass as bass  # noqa: F401
    import concourse.tile as tile
    import concourse.mybir as mybir
    from concourse import bacc
    from concourse.masks import make_identity
    from contextlib import ExitStack

    f32 = mybir.dt.float32
    f32r = mybir.dt.float32r
    bf16 = mybir.dt.bfloat16

    nc = bacc.Bacc("TRN2", target_bir_lowering=False, debug=False,
                   num_devices=NCORES)

    x_d = nc.dram_tensor("x", [S, D], f32, kind="ExternalInput").ap()
    wk_d = nc.dram_tensor("Wk", [D, D], f32, kind="ExternalInput").ap()
    wo_d = nc.dram_tensor("Wo", [D, D], f32, kind="ExternalInput").ap()
    out_d = nc.dram_tensor("out", [SH, D], f32, kind="ExternalOutput").ap()

    NSC = S // P           # 16 sequence chunks
    NDC = D // P           # 4 feature chunks
    NQB = SH // 512        # 2 query blocks of 512
    QB = 512

    import contextlib
    with tile.TileContext(nc) as tc:
        loop_cm = tc.For_i(0, repeat, 1) if repeat > 1 else contextlib.nullcontext()
        with loop_cm, ExitStack() as ctx:
            consts = ctx.enter_context(tc.tile_pool(name="consts", bufs=1))
            kpool = ctx.enter_context(tc.tile_pool(name="kpool", bufs=1))
            _ebufs = 5 if "ct" in mode else (3 if "e3" in mode else 4)
            epool = ctx.enter_context(
                tc.tile_pool(name="epool", bufs=_ebufs))
            vpool = ctx.enter_context(tc.tile_pool(name="vpool", bufs=1))
            opool = ctx.enter_context(
                tc.tile_pool(name="opool", bufs=(3 if "x4" in mode else 2)))
            # psum: tag A = 4 banks x1, tag B = 2 banks x2  -> 8 banks total
            ps = ctx.enter_context(tc.tile_pool(name="ps", bufs=1, space="PSUM"))

            bigspan = "bigspan" in mode

            def spanA(dtype=None):
                return ps.tile([P, 4, 512], dtype or f32, tag="A",
                               bufs=(2 if bigspan else 1), name="spA")

            def spanB(shape=None, name="spB"):
                if bigspan:
                    return ps.tile(shape or [P, 2, 512], f32, tag="A", bufs=2,
                                   name=name)
                return ps.tile(shape or [P, 2, 512], f32, tag="B", bufs=2,
                               name=name)

            ident = consts.tile([P, P], f32)
            make_identity(nc, ident[:])

            ones1x64f = consts.tile([1, 64], f32)
            nc.gpsimd.memset(ones1x64f[:], 1.0)
            ones1x64 = consts.tile([1, 64], f32r)
            nc.vector.tensor_copy(ones1x64[:], ones1x64f[:])
            ones_bf = consts.tile([P, 1], bf16)
            nc.gpsimd.memset(ones_bf[:], 1.0)
            ident_bf = consts.tile([P, P], bf16)
            nc.vector.tensor_copy(ident_bf[:], ident[:])

            woT = consts.tile([P, NDC, 512], f32r)      # [d', e]
            k_bf = kpool.tile([P, NSC, 512], bf16)      # K [s, e]
            kones = (None if "ct" in mode else
                     kpool.tile([P, NSC, H, HD + 1], bf16))
            khT = kpool.tile([P, NDC, S], bf16)         # K^T [e, s]
            wvt = vpool.tile([P, NDC, SH], f32r)        # wVT (norm in place)

            # ---- phase 0/1: weights, x -> xT (PE transposes) -> K proj -----
            with tc.tile_pool(name="stage", bufs=1) as stage:
                wk_sb = stage.tile([P, NDC, 512], f32, tag="w")
                nc.sync.dma_start(wk_sb[:], wk_d.rearrange("(eo p) d -> p eo d", p=P))
                wkT_r = consts.tile([P, NDC, 512], f32r)
                sp = spanA()
                for dc in range(NDC):
                    for eo in range(NDC):
                        nc.tensor.transpose(
                            sp[:, dc, eo * P:(eo + 1) * P],
                            wk_sb[:, eo, dc * P:(dc + 1) * P], ident[:])
                nc.scalar.copy(wkT_r[:], sp[:])

                # x: 8 groups of 2 seq-chunks; PE transpose -> fp32r Kproj
                for g in range(8):
                    g0 = g * 2
                    x_g = stage.tile([P, 2, 512], f32, tag="x",
                                     bufs=(4 if "x4" in mode else 3),
                                     name="x_g")
                    nc.sync.dma_start(
                        x_g[:, 0:2, :],
                        x_d[g0 * P:(g0 + 2) * P, :].rearrange(
                            "(two p) d -> p two d", p=P))
                    spt = spanB([P, 4, 256], name="sptr")
                    for i in range(2):
                        for dc in range(NDC):
                            nc.tensor.transpose(
                                spt[:, dc, i * P:(i + 1) * P],
                                x_g[:, i, dc * P:(dc + 1) * P], ident[:])
                    xT_g = stage.tile([P, NDC, 256], f32r, tag="xT", bufs=2,
                                      name="xT_g")
                    nc.scalar.copy(xT_g[:], spt[:])

                    spk = spanB(name="spkp")
                    for i in range(2):
                        for dc in range(NDC):
                            nc.tensor.matmul(
                                spk[:, i, :],
                                xT_g[:, dc, i * P:(i + 1) * P],
                                wkT_r[:, dc, :],
                                start=(dc == 0), stop=(dc == NDC - 1))
                    nc.vector.tensor_copy(k_bf[:, g0:g0 + 2, :], spk[:, 0:2, :])
                    if kones is not None:
                        nc.vector.tensor_copy(
                            kones[:, g0:g0 + 2, :, 0:HD],
                            spk[:, 0:2, :].rearrange("p g (h e) -> p g h e",
                                                     h=H))
                    if "pekt" in mode:
                        # khT via PE transposes (bf16): 8 tiles -> one A span
                        spkt = spanA(bf16)
                        for i in range(2):
                            sc = g0 + i
                            for ec in range(NDC):
                                nc.tensor.transpose(
                                    spkt[:, ec, i * P:(i + 1) * P],
                                    k_bf[:, sc, ec * P:(ec + 1) * P],
                                    ident_bf[:])
                        nc.vector.tensor_copy(
                            khT[:, :, g0 * P:(g0 + 2) * P],
                            spkt[:, 0:NDC, 0:2 * P])

                # Wo DMA early (keeps all DMACopies before the XBAR
                # cluster); its PE transposes run after the khT transposes
                # so head-0 scores can start as soon as khT chunks land.
                wo_sb = stage.tile([P, NDC, 512], f32, tag="w")
                nc.sync.dma_start(wo_sb[:], wo_d.rearrange("(eo p) d -> p eo d", p=P))

                if "pekt" not in mode:
                    # khT via XBAR transposes, one cluster after all DMAs
                    for sc in range(NSC):
                        nc.sync.dma_start_transpose(
                            khT[:, :, sc * P:(sc + 1) * P], k_bf[:, sc, :])

                # Wo: transpose via PE (fp32 -> fp32r), span A
                sp = spanA()
                for dc in range(NDC):
                    for eo in range(NDC):
                        nc.tensor.transpose(
                            sp[:, dc, eo * P:(eo + 1) * P],
                            wo_sb[:, eo, dc * P:(dc + 1) * P], ident[:])
                nc.scalar.copy(woT[:], sp[:])

            if kones is not None:
                nc.gpsimd.memset(kones[:, :, :, HD:HD + 1], 1.0)

            if "phase0" in mode:
                # consume everything so DCE cannot strip phase 0/1 work
                with tc.tile_pool(name="sink", bufs=1, space="DRAM") as sink:
                    snk1 = sink.tile([P, NSC, 512], bf16, name="snk1")
                    nc.sync.dma_start(snk1[:], k_bf[:])
                    snk2 = sink.tile([P, NDC, S], bf16, name="snk2")
                    nc.sync.dma_start(snk2[:], khT[:])
                    snk3 = sink.tile([P, NSC, H, HD + 1], bf16, name="snk3")
                    nc.sync.dma_start(snk3[:], kones[:])
                    snk4 = sink.tile([P, NDC, 512], f32, name="snk4")
                    nc.sync.dma_start(snk4[:], woT[:].bitcast(f32))
                o_sb0 = opool.tile([P, 2, 512], f32, tag="osb", name="o_sb0")
                nc.vector.tensor_copy(o_sb0[:, 0, :], khT[:, 0, 0:512])
                nc.vector.tensor_copy(o_sb0[:, 1, :], k_bf[:, 0, :])
                nc.sync.dma_start(
                    out_d[0:2 * P, :].rearrange("(two p) d -> p two d", p=P),
                    o_sb0[:])

            # ---- head loop (qb outer), software-pipelined PV ----------------
            if bigspan:
                kc_groups = [(0, "A"), (4, "A"), (8, "A"), (12, "A")]
            else:
                kc_groups = [(0, "A"), (4, "B"), (6, "B"), (8, "A"),
                             (12, "B"), (14, "B")]

            def emit_pv(h, qb, e_t):
                hp = (h % 2) * HD
                ec = h // 2
                pv = spanB([HD + 1, 512], name="pv")
                for kc in range(NSC):
                    nc.tensor.matmul(
                        pv[:], kones[:, kc, h, :], e_t[:, kc, :],
                        start=(kc == 0), stop=(kc == NSC - 1))
                nc.vector.tensor_copy(
                    wvt[hp:hp + HD, ec, qb * QB:(qb + 1) * QB], pv[0:HD, :])
                recip_t = vpool.tile([1, 512], f32r, tag="recip", bufs=4,
                                     name="recip_t")
                with nc.allow_low_precision(reason="fp32r recip is fine"):
                    nc.vector.reciprocal(recip_t[:], pv[HD:HD + 1, :])
                bc = spanB([HD, 512], name="bc")
                nc.tensor.matmul(
                    bc[:], ones1x64[:], recip_t[:], start=True, stop=True)
                nc.vector.tensor_tensor(
                    wvt[hp:hp + HD, ec, qb * QB:(qb + 1) * QB],
                    wvt[hp:hp + HD, ec, qb * QB:(qb + 1) * QB],
                    bc[:], mybir.AluOpType.mult)

            def emit_outproj(qc0):
                # two q-chunks of 128 per pass, psum in a B slot
                po = spanB(name="po")
                for j in range(2):
                    qc = qc0 + j
                    for dc in range(NDC):
                        nc.tensor.matmul(
                            po[:, j, :],
                            wvt[:, dc, qc * P:(qc + 1) * P],
                            woT[:, dc, :],
                            start=(dc == 0), stop=(dc == NDC - 1))
                o_sb = opool.tile([P, 2, 512], f32, tag="osb", name="o_sb")
                nc.vector.tensor_copy(o_sb[:], po[:])
                nc.sync.dma_start(
                    out_d[qc0 * P:(qc0 + 2) * P, :].rearrange(
                        "(two p) d -> p two d", p=P),
                    o_sb[:])

            def emit_pv_pair(j, qb, e_lo, e_hi):
                # heads (2j, 2j+1) concurrently via PE column tiling
                pv = spanB([P, 512], name="pvp")
                for kc in range(NSC):
                    nc.tensor.matmul(
                        pv[0:HD, :],
                        k_bf[:, kc, (2 * j) * HD:(2 * j + 1) * HD],
                        e_lo[:, kc, :],
                        start=(kc == 0), stop=(kc == NSC - 1),
                        tile_position=(0, 0))
                    nc.tensor.matmul(
                        pv[HD:2 * HD, :],
                        k_bf[:, kc, (2 * j + 1) * HD:(2 * j + 2) * HD],
                        e_hi[:, kc, :],
                        start=(kc == 0), stop=(kc == NSC - 1),
                        tile_position=(0, HD))
                nc.vector.tensor_copy(
                    wvt[:, j, qb * QB:(qb + 1) * QB], pv[:])

            def emit_rs_quad(g, qb, e_ts):
                # rowsums of heads 4g..4g+3 via 4-way column tiling (M=1)
                rs = spanB([97, 512], name="rs")
                for kc in range(NSC):
                    for hi in range(4):
                        nc.tensor.matmul(
                            rs[32 * hi:32 * hi + 1, :],
                            ones_bf[:, 0:1],
                            e_ts[hi][:, kc, :],
                            start=(kc == 0), stop=(kc == NSC - 1),
                            tile_position=(0, 32 * hi))
                for hi in range(4):
                    h = 4 * g + hi
                    hp = (h % 2) * HD
                    ec = h // 2
                    recip_t = vpool.tile([1, 512], f32r, tag="recip", bufs=4,
                                         name="recip_t")
                    with nc.allow_low_precision(reason="fp32r recip is fine"):
                        nc.vector.reciprocal(recip_t[:],
                                             rs[32 * hi:32 * hi + 1, :])
                    bc = spanB([HD, 512], name="bc")
                    nc.tensor.matmul(
                        bc[:], ones1x64[:], recip_t[:], start=True, stop=True)
                    nc.vector.tensor_tensor(
                        wvt[hp:hp + HD, ec, qb * QB:(qb + 1) * QB],
                        wvt[hp:hp + HD, ec, qb * QB:(qb + 1) * QB],
                        bc[:], mybir.AluOpType.mult)

            if "ct" in mode:
                for qb in range(NQB):
                    quad = []
                    for h in range(H):
                        hp = (h % 2) * HD
                        ec = h // 2
                        e_t = epool.tile([P, NSC, 512], bf16, tag="E",
                                         name="e_t")
                        for g0, kind in kc_groups:
                            gn = 4 if kind == "A" else 2
                            sp = spanA() if kind == "A" else spanB()
                            for i in range(gn):
                                kc = g0 + i
                                nc.tensor.matmul(
                                    sp[:, i, :],
                                    khT[hp:hp + HD, ec, kc * P:(kc + 1) * P],
                                    khT[hp:hp + HD, ec, qb * QB:(qb + 1) * QB],
                                    start=True, stop=True)
                            nc.scalar.activation(
                                e_t[:, g0:g0 + gn, :], sp[:, 0:gn, :],
                                mybir.ActivationFunctionType.Exp, scale=SCALE)
                        quad.append(e_t)
                        if h % 2 == 1:
                            emit_pv_pair(h // 2, qb, quad[-2], quad[-1])
                        if h % 4 == 3:
                            emit_rs_quad(h // 4, qb, quad)
                            quad = []
                    for qc0 in range(qb * 4, qb * 4 + 4, 2):
                        emit_outproj(qc0)
            else:
                pending = None
                backlog = []
                for qb in range(NQB if "phase0" not in mode else 0):
                    for h in range(H):
                        hp = (h % 2) * HD
                        ec = h // 2
                        e_t = epool.tile([P, NSC, 512], bf16, tag="E", name="e_t")
                        for g0, kind in kc_groups:
                            gn = 4 if kind == "A" else 2
                            sp = spanA() if kind == "A" else spanB()
                            for i in range(gn):
                                kc = g0 + i
                                nc.tensor.matmul(
                                    sp[:, i, :],
                                    khT[hp:hp + HD, ec, kc * P:(kc + 1) * P],
                                    khT[hp:hp + HD, ec, qb * QB:(qb + 1) * QB],
                                    start=True, stop=True)
                            nc.scalar.activation(
                                e_t[:, g0:g0 + gn, :], sp[:, 0:gn, :],
                                mybir.ActivationFunctionType.Exp, scale=SCALE)
                        if pending is not None and "full" in mode:
                            emit_pv(*pending)
                            if pending[0] == H - 1:
                                backlog.extend(
                                    range(pending[1] * 4,
                                          pending[1] * 4 + 4, 2))
                            if (backlog and "spread" in mode
                                    and pending[0] % 2 == 1):
                                emit_outproj(backlog.pop(0))
                            elif backlog and "spread" not in mode:
                                while backlog:
                                    emit_outproj(backlog.pop(0))
                        pending = (h, qb, e_t)
                if "full" in mode:
                    emit_pv(*pending)
                    backlog.extend(
                        range(pending[1] * 4, pending[1] * 4 + 4, 2))
                    for qc0 in backlog:
                        emit_outproj(qc0)

    nc.compile()
    return nc


def _get_nc(repeat: int = 1, mode: str = "full"):
    key = ("nc", repeat, mode)
    if key not in _CACHE:
        _CACHE[key] = _build_nc(repeat, mode)
    return _CACHE[key]


def _make_in_maps(x, Wk, Wo):
    in_maps = []
    for c in range(NCORES):
        b, half = c // 2, c % 2
        xb = x[b]
        if half:
            xb = np.roll(xb, -SH, axis=0)
        in_maps.append({"x": np.ascontiguousarray(xb), "Wk": Wk, "Wo": Wo})
    return in_maps


def kernel(x: np.ndarray, Wk: np.ndarray, Wo: np.ndarray, _trace=False):
    from concourse import bass_utils

    nc = _get_nc()
    x = np.asarray(x, dtype=np.float32)
    Wk = np.ascontiguousarray(np.asarray(Wk, dtype=np.float32))
    Wo = np.ascontiguousarray(np.asarray(Wo, dtype=np.float32))

    in_maps = _make_in_maps(x, Wk, Wo)

    res = bass_utils.run_bass_kernel_spmd(
        nc, in_maps, core_ids=list(range(NCORES)), trace=_trace)

    out = np.empty((B, S, D), dtype=np.float32)
    for c in range(NCORES):
        b, half = c // 2, c % 2
        out[b, half * SH:(half + 1) * SH] = res.results[c]["out"]
    if _trace:
        _CACHE["last_results"] = res
    return out



# revision 8
# speedup vs baseline: 1.0860x; 1.0860x over previous
"""Trainium2 Bass kernel for MultiHeadSelfAttention (K-only variant).

Math (per batch b):
    K  = x @ Wk.T;  Kh = heads(K)
    S_h = Kh @ Kh.T / sqrt(D);  P_h = softmax(S_h)
    wV_h = P_h @ Kh  (V == K);  out = concat_h(wV) @ Wo.T

Sharding (8 cores): core c handles batch c//2 and query-half c%2 with all
heads.  The query half is selected by rolling x on the host so each core
always computes queries 0:S//2 of its (rolled) sequence; softmax over keys
is order-invariant so rolling the key axis is harmless.

Host pre-transposes x, Wk, Wo (xT=[D,S], WkT=[D,D], WoT=[D,D]) so the
device does zero layout transposes on the d-contraction operands.

Per-core pipeline (one SPMD NEFF), engine-balanced around ScalarE exp
(~110us of pure exp work/core is the floor):
    K       = xT.T @ WkT (f32r matmuls, fp32 psum) -> k_bf (bf16)
    khT     = XBAR-DMA-transpose(k_bf)     [e, s] bf16 (ACT hwdge queue)
    scores  : per (pair, kc): two row-tiled bf16 matmuls (K=64 heads a/b)
              -> one 2-bank psum span; ScalarE exp (free 1024) -> e tile
    PV      : col-tiled pair matmuls (M=64+64) accumulate over kc
    rowsums : 4-way col-tiled M=1 ones-matmuls (quad per head group)
    recip -> pair-broadcast matmul -> tensor_tensor normalize into wvt
    out     = wvt.T @ WoT (f32r), per 128-query chunk, DMA out
"""

import sys

if "/opt/trn_rl_repo" not in sys.path:
    sys.path.insert(0, "/opt/trn_rl_repo")

import numpy as np

B, S, D = 4, 2048, 512
H = 8
HD = D // H            # 64
P = 128
SH = S // 2            # query half per core = 1024
NCORES = 8
SCALE = 1.0 / np.sqrt(D)

_CACHE = {}


def _build_nc(repeat: int = 1, mode: str = "v2"):
    import concourse.bass as bass  # noqa: F401
    import concourse.tile as tile
    import concourse.mybir as mybir
    from concourse import bacc
    from contextlib import ExitStack
    import contextlib

    f32 = mybir.dt.float32
    f32r = mybir.dt.float32r
    bf16 = mybir.dt.bfloat16
    Exp = mybir.ActivationFunctionType.Exp
    mult = mybir.AluOpType.mult

    nc = bacc.Bacc("TRN2", target_bir_lowering=False, debug=False,
                   num_devices=NCORES)

    xT_d = nc.dram_tensor("xT", [D, S], bf16, kind="ExternalInput").ap()
    wkT_d = nc.dram_tensor("WkT", [D, D], bf16, kind="ExternalInput").ap()
    woT_d = nc.dram_tensor("WoT", [D, D], bf16, kind="ExternalInput").ap()
    out_d = nc.dram_tensor("out", [SH, D], f32, kind="ExternalOutput").ap()

    NSC = S // P           # 16 key chunks (also rounds per pair)
    NDC = D // P           # 4 feature chunks
    NQB = SH // 512        # 2 query blocks of 512
    QB = 512
    NPAIR = 4              # head pairs per qb
    NU = NQB * NPAIR       # 8 pair-units

    with tile.TileContext(nc) as tc:
        loop_cm = tc.For_i(0, repeat, 1) if repeat > 1 else \
            contextlib.nullcontext()
        with loop_cm, ExitStack() as ctx:
            consts = ctx.enter_context(tc.tile_pool(name="consts", bufs=1))
            kpool = ctx.enter_context(tc.tile_pool(name="kpool", bufs=1))
            epool = ctx.enter_context(tc.tile_pool(name="epool", bufs=3))
            vpool = ctx.enter_context(tc.tile_pool(name="vpool", bufs=1))
            stage = ctx.enter_context(tc.tile_pool(name="stage", bufs=1))
            ps = ctx.enter_context(
                tc.tile_pool(name="ps", bufs=1, space="PSUM"))

            ones_bf = consts.tile([P, 1], bf16)
            nc.gpsimd.memset(ones_bf[:], 1.0)
            ones64f = consts.tile([1, HD], f32)
            nc.gpsimd.memset(ones64f[:], 1.0)
            ones64 = consts.tile([1, HD], f32r)
            nc.vector.tensor_copy(ones64[:], ones64f[:])

            wkT_sb = kpool.tile([P, NDC, D], bf16)      # [d', dc, e]
            woT_sb = kpool.tile([P, NDC, D], bf16)
            k_bf = kpool.tile([P, NSC, D], bf16)        # K [s, e]
            khT = kpool.tile([P, NDC, S], bf16)         # K^T [e, s]
            wvt = kpool.tile([P, NDC, SH], bf16)        # normalized wV^T

            nc.sync.dma_start(
                wkT_sb[:], wkT_d.rearrange("(dc p) e -> p dc e", p=P))
            nc.sync.dma_start(
                woT_sb[:], woT_d.rearrange("(dc p) e -> p dc e", p=P))

            # ---------------- emission helpers -------------------------
            pair_state = [None] * NU     # u -> dict(e, pv, recips)

            def emit_group(g):
                """x^T chunk load -> K projection -> k_bf -> khT XBAR."""
                xT_g = stage.tile([P, NDC, 2 * P], bf16, tag="x", bufs=3,
                                  name="xT_g")
                nc.sync.dma_start(
                    xT_g[:],
                    xT_d[:, g * 2 * P:(g + 1) * 2 * P].rearrange(
                        "(dc p) s -> p dc s", p=P))
                kp = ps.tile([P, 2, QB], f32, tag="S", bufs=2, name="kp")
                for i in range(2):
                    for dc in range(NDC):
                        nc.tensor.matmul(
                            kp[:, i, :],
                            xT_g[:, dc, i * P:(i + 1) * P],
                            wkT_sb[:, dc, :],
                            start=(dc == 0), stop=(dc == NDC - 1))
                nc.vector.tensor_copy(k_bf[:, 2 * g:2 * g + 2, :], kp[:])
                for i in range(2):
                    sc = 2 * g + i
                    nc.scalar.dma_start_transpose(
                        khT[:, :, sc * P:(sc + 1) * P], k_bf[:, sc, :])

            def pair_qb(u):
                return u // NPAIR

            def pair_p(u):
                return u % NPAIR

            def emit_sround(u, r):
                """Scores pair matmuls for key chunk r + exp into e tile."""
                qb, p = pair_qb(u), pair_p(u)
                st = pair_state[u]
                if st is None:
                    st = pair_state[u] = {
                        "e": epool.tile([P, NSC, 2, QB], bf16, tag="E",
                                        name="e_t"),
                        "pv": None, "recip": None,
                    }
                sp = ps.tile([P, 2, QB], f32, tag="S", bufs=2, name="sp")
                qs = slice(qb * QB, (qb + 1) * QB)
                for j in range(2):
                    hp = j * HD
                    nc.tensor.matmul(
                        sp[:, j, :],
                        khT[hp:hp + HD, p, r * P:(r + 1) * P],
                        khT[hp:hp + HD, p, qs],
                        start=True, stop=True)
                nc.scalar.activation(
                    st["e"][:, r, :, :], sp[:], Exp, scale=SCALE)

            def emit_pv(u, kc):
                """PV pair (col-tiled) accumulation for key chunk kc."""
                p = pair_p(u)
                st = pair_state[u]
                if st["pv"] is None:
                    st["pv"] = ps.tile([P, QB], f32, tag="PV", bufs=2,
                                       name="pv")
                pv = st["pv"]
                for j in range(2):
                    nc.tensor.matmul(
                        pv[j * HD:(j + 1) * HD, :],
                        k_bf[:, kc, p * P + j * HD:p * P + (j + 1) * HD],
                        st["e"][:, kc, j, :],
                        start=(kc == 0), stop=(kc == NSC - 1),
                        tile_position=(0, j * HD))

            quad_rs = [None] * (NU // 2)

            def emit_quad(gq, kc):
                """Rowsums of group gq's 4 heads for key chunk kc."""
                if quad_rs[gq] is None:
                    quad_rs[gq] = ps.tile([P, QB], f32, tag="RS", bufs=1,
                                          name="rs")
                rs = quad_rs[gq]
                for hi in range(4):
                    st = pair_state[2 * gq + hi // 2]
                    nc.tensor.matmul(
                        rs[32 * hi:32 * hi + 1, :],
                        ones_bf[:, 0:1],
                        st["e"][:, kc, hi % 2, :],
                        start=(kc == 0), stop=(kc == NSC - 1),
                        tile_position=(0, 32 * hi))

            def emit_norm(gq):
                """recip -> pair-broadcast -> normalize into wvt; frees
                the group's pv/rs/e psum+sbuf resources."""
                qb = (2 * gq) // NPAIR
                rs = quad_rs[gq]
                qs = slice(qb * QB, (qb + 1) * QB)
                for hi in range(4):
                    st = pair_state[2 * gq + hi // 2]
                    if st["recip"] is None:
                        st["recip"] = [None, None]
                    rt = vpool.tile([1, QB], f32r, tag="recip", bufs=8,
                                    name="recip_t")
                    with nc.allow_low_precision(reason="f32r recip ok"):
                        nc.vector.reciprocal(
                            rt[:], rs[32 * hi:32 * hi + 1, :])
                    st["recip"][hi % 2] = rt
                for up in (2 * gq, 2 * gq + 1):
                    st = pair_state[up]
                    p = pair_p(up)
                    for j in range(2):
                        hp = j * HD
                        bc = ps.tile([HD, QB], f32, tag="C", bufs=1,
                                     name="bc")
                        nc.tensor.matmul(
                            bc[:], ones64[:], st["recip"][j][:],
                            start=True, stop=True)
                        # DVE reads at most one PSUM operand: stage in SBUF.
                        bc_sb = vpool.tile([HD, QB], bf16, tag="bcs",
                                           bufs=4, name="bc_sb")
                        nc.vector.tensor_copy(bc_sb[:], bc[:])
                        nc.vector.tensor_tensor(
                            wvt[hp:hp + HD, p, qs],
                            st["pv"][hp:hp + HD, :], bc_sb[:], mult)
                quad_rs[gq] = None

            def emit_outproj(qc):
                """Output projection for query chunk qc (128 queries)."""
                po = ps.tile([P, QB], f32, tag="C", bufs=1, name="po")
                for dc in range(NDC):
                    nc.tensor.matmul(
                        po[:],
                        wvt[:, dc, qc * P:(qc + 1) * P],
                        woT_sb[:, dc, :],
                        start=(dc == 0), stop=(dc == NDC - 1))
                o_sb = vpool.tile([P, QB], f32, tag="osb", bufs=2,
                                  name="o_sb")
                nc.vector.tensor_copy(o_sb[:], po[:])
                nc.sync.dma_start(
                    out_d[qc * P:(qc + 1) * P, :], o_sb[:])

            # ---------------- master schedule --------------------------
            # Phase 1 with pair-0 score rounds interleaved.
            for g in range(8):
                emit_group(g)
                if g >= 2:
                    emit_sround(0, 2 * (g - 2))
                    emit_sround(0, 2 * (g - 2) + 1)
            for r in range(12, NSC):
                emit_sround(0, r)

            # Steady-state pair-unit rounds.
            outproj_backlog = []
            for u in range(1, NU):
                for r in range(NSC):
                    emit_sround(u, r)
                    if r == 0 and u >= 2:
                        emit_pv(u - 1, NSC - 1)
                        if u % 2 == 0:
                            emit_quad((u - 2) // 2, NSC - 1)
                    if r == 1 and u % 2 == 0 and u >= 2:
                        # Must precede emit_pv(u, 0): frees the pv slot the
                        # new pair is about to claim (PE-queue FIFO order).
                        emit_norm((u - 2) // 2)
                        if (u - 2) // 2 == 1:
                            # qb0 fully normalized (groups 0,1)
                            outproj_backlog = list(range(4))
                    if u == 1:
                        emit_pv(0, r)
                    if r >= 1:
                        emit_pv(u, r - 1)
                    if u % 2 == 1 and r >= 1:
                        emit_quad(u // 2, r - 1)
                    if outproj_backlog and r % 4 == 2:
                        emit_outproj(outproj_backlog.pop(0))
            # Tail: finish last pair / group / qb1 projection.
            emit_pv(NU - 1, NSC - 1)
            emit_quad(NU // 2 - 1, NSC - 1)
            emit_norm(NU // 2 - 1)
            for qc in range(4, 8):
                emit_outproj(qc)

    nc.compile()
    return nc


def _get_nc(repeat: int = 1, mode: str = "v2"):
    key = ("nc", repeat, mode)
    if key not in _CACHE:
        _CACHE[key] = _build_nc(repeat, mode)
    return _CACHE[key]


def _make_in_maps(x, Wk, Wo):
    import ml_dtypes

    bf = ml_dtypes.bfloat16
    wkT = np.ascontiguousarray(Wk.T).astype(bf)
    woT = np.ascontiguousarray(Wo.T).astype(bf)
    in_maps = []
    for c in range(NCORES):
        b, half = c // 2, c % 2
        xb = x[b]
        if half:
            xb = np.roll(xb, -SH, axis=0)
        in_maps.append({"xT": np.ascontiguousarray(xb.T).astype(bf),
                        "WkT": wkT, "WoT": woT})
    return in_maps


def kernel(x: np.ndarray, Wk: np.ndarray, Wo: np.ndarray, _trace=False):
    from concourse import bass_utils

    nc = _get_nc()
    x = np.asarray(x, dtype=np.float32)
    Wk = np.ascontiguousarray(np.asarray(Wk, dtype=np.float32))
    Wo = np.ascontiguousarray(np.asarray(Wo, dtype=np.float32))

    in_maps = _make_in_maps(x, Wk, Wo)

    res = bass_utils.run_bass_kernel_spmd(
        nc, in_maps, core_ids=list(range(NCORES)), trace=_trace)

    out = np.empty((B, S, D), dtype=np.float32)
    for c in range(NCORES):
        b, half = c // 2, c % 2
        out[b, half * SH:(half + 1) * SH] = res.results[c]["out"]
    if _trace:
        _CACHE["last_results"] = res
    return out
